# revision 25
# baseline (speedup 1.0000x reference)
"""AttentionBlock3D Trainium2 kernel.

Module: GroupNorm(8 groups) -> 1x1x1 conv QKV -> 4-head attention over
N=4096 spatial positions (head_dim 64) -> 1x1x1 conv proj -> residual.
Shapes: x [2, 256, 16, 16, 16] f32.

Sharding (8 cores): batch (2) x query-range (4 chunks of 1024 rows).
Host rotates each core's x columns so its 1024 query columns come first
(GroupNorm stats and the attention key-sum are order-invariant), and
ships x as bf16 -- so there is no separate query slice DMA and no
on-device normalize pass.

Per core:
  - GroupNorm stats from chunked bn_stats overlapping the x DMA;
    rstd = exp(-0.5*ln(var+eps)) so Ln/Exp share one ACT table set.
  - The GN affine (s_c, t_c) is folded into the QKV weights on device:
    W' = W * s (per input channel), bias' = b + W @ t (tiny matmuls),
    so QKV matmuls read raw bf16 x directly.
  - k, v for ALL 4096 keys (all heads), q only for its 1024 rows.
  - Attention is software-pipelined: the PE emits scores(gi) before
    AV(gi-1), so matmuls stream continuously (HAM stays warm) while
    ACT (exact exp) and DVE (Schraudolph bf16 bit-trick exp) run
    concurrently on alternating half-head score tiles.
  - Softmax is unnormalized; the denominator comes from ones-columns in
    the AV matmul (free: matmul cost is N-cycles regardless of M) and is
    divided out after attention*V.
  - proj + bias + residual for its [256, 1024] output slice.
Gather on host is pure concatenation.

Layouts on device (per core):
  x  [C=256, N] bf16 -> 2 channel-tiles of [128, N]
  k_sb[pair]      [128, 4096] bf16: partitions = [head 2p (64); 2p+1]
  q_sb[pair]      [128, 1024] bf16: same head-pair packing
  vT2_sb          [128, 32*512] bf16: partitions = key rows m; per key
                  tile mt four 128-col blocks (hp, hh), hh=0 [v|ones],
                  hh=1 [ones|v]
  scores^T        PSUM [m 128, n 512] via row-tiled (K=64) matmul pairs
  attention out   acc[hh] [128, 512]: o and its softmax denominator land
                  in complementary partition halves of the same bank
"""

import math
import numpy as np

B = 2
C = 256
NH = 4
GROUPS = 8
EPS = 1e-5
N = 16 * 16 * 16  # 4096
HD = C // NH      # 64
NQ = N // 4       # 1024 query rows per core
NCORES = 8
CT = 2            # channel tiles of 128
MT = N // 128     # 32 key tiles
SCALE = HD ** -0.5
# Schraudolph bf16 exp: bits_i16(round(A*x + B)) viewed as bf16 ~= exp(x)
EXP_A = SCALE * 128.0 / math.log(2.0)
EXP_B = 127.0 * 128.0 - 0.0430 * 128.0


def _build_nc(finalize=True):
    import concourse.bacc as bacc
    import concourse.bass as bass
    import concourse.mybir as mybir
    from concourse.tile import TileContext

    f32 = mybir.dt.float32
    bf16 = mybir.dt.bfloat16
    fp8 = mybir.dt.float8e4
    DR = mybir.MatmulPerfMode.DoubleRow
    Alu = mybir.AluOpType
    AF = mybir.ActivationFunctionType

    nc = bacc.Bacc("TRN2", debug=False)

    xf = nc.dram_tensor("xf", [C, N], bf16, kind="ExternalInput").ap()
    wT = nc.dram_tensor("wT", [C, 3 * C], bf16, kind="ExternalInput").ap()
    pT = nc.dram_tensor("pT", [C, C], bf16, kind="ExternalInput").ap()
    qb = nc.dram_tensor("qb", [128, 4], f32, kind="ExternalInput").ap()
    pb = nc.dram_tensor("pb", [C], f32, kind="ExternalInput").ap()
    nw = nc.dram_tensor("nw", [C], f32, kind="ExternalInput").ap()
    nb = nc.dram_tensor("nb", [C], f32, kind="ExternalInput").ap()
    Gm = nc.dram_tensor("Gm", [CT, 128, GROUPS], f32, kind="ExternalInput").ap()
    Pm = nc.dram_tensor("Pm", [CT, GROUPS, 128], f32, kind="ExternalInput").ap()
    idm = nc.dram_tensor("idm", [128, 128], bf16, kind="ExternalInput").ap()
    y = nc.dram_tensor("y", [C, NQ], f32, kind="ExternalOutput").ap()

    with TileContext(nc) as tc:
        import contextlib

        est = contextlib.ExitStack()
        with est:
            singles = est.enter_context(tc.tile_pool(name="singles", bufs=1))

            # ---------- persistent SBUF tiles ----------
            x_sb = [singles.tile([128, N], bf16, tag=f"x{ct}", name=f"x{ct}") for ct in range(CT)]
            wT_sb = [singles.tile([128, 3 * C], bf16, tag=f"wT{ct}", name=f"wT{ct}") for ct in range(CT)]
            wTs_sb = [singles.tile([128, 3 * C], bf16, tag=f"wTs{ct}", name=f"wTs{ct}") for ct in range(CT)]
            pT_sb = [singles.tile([128, C], bf16, tag=f"pT{ct}", name=f"pT{ct}") for ct in range(CT)]
            k_sb = [singles.tile([128, N], bf16, tag=f"k{p}", name=f"k{p}") for p in range(2)]
            q_sb = [singles.tile([128, NQ], bf16, tag=f"q{p}", name=f"q{p}") for p in range(2)]
            # vT2: [p, mt(32):512, hp(2):256, hh(2):128, d(128):1]
            # d = [v(64)|ones(64)] for hh=0, [ones(64)|v(64)] for hh=1
            vT2_sb = singles.tile([128, MT * 512], bf16, tag="vT2", name="vT2")
            qb_sb = singles.tile([128, 4], f32, tag="qb", name="qb")
            qb2_sb = singles.tile([128, 4], f32, tag="qb2", name="qb2")
            pb_sb = singles.tile([128, 2], f32, tag="pb", name="pb")
            pb2_sb = singles.tile([128, 2], f32, tag="pb2", name="pb2")
            nw_sb = singles.tile([128, CT], f32, tag="nw", name="nw")
            nb_sb = singles.tile([128, CT], f32, tag="nb", name="nb")
            G_sb = [singles.tile([128, GROUPS], f32, tag=f"G{ct}", name=f"G{ct}") for ct in range(CT)]
            P_sb = [singles.tile([GROUPS, 128], f32, tag=f"P{ct}", name=f"P{ct}") for ct in range(CT)]
            eps_sb = singles.tile([128, 1], f32, tag="eps", name="eps")
            s_sb = [singles.tile([128, 1], f32, tag=f"s{ct}", name=f"s{ct}") for ct in range(CT)]
            t16_sb = singles.tile([128, CT], bf16, tag="t16", name="t16")
            u16_sb = singles.tile([128, 2], bf16, tag="u16", name="u16")
            mr_sb = singles.tile([GROUPS, 2], f32, tag="mr", name="mr")
            rs_sb = singles.tile([GROUPS, 1], f32, tag="rs", name="rs")
            dm_sb = singles.tile([GROUPS, 1], f32, tag="dm", name="dm")
            warm_sb = singles.tile([128, 512], bf16, tag="warm", name="warm")
            scr_sb = singles.tile([128, 1024], f32, tag="scr", name="scr")
            id_sb = singles.tile([128, 128], bf16, tag="idm", name="idm")

            def v2_view(off, dims):
                return bass.AP(
                    tensor=vT2_sb.tensor,
                    offset=vT2_sb.offset + off,
                    ap=[list(vT2_sb.ap[0])] + [list(d) for d in dims],
                )

            # ---------- t=0: x DMA first (8 parallel queues, issued from
            # two engines so descriptor writes don't serialize), then weights
            nc.vector.memset(warm_sb, 0.0)
            nc.vector.memset(eps_sb, EPS)
            # x in 16 [128, 512] chunks over the three DMA-capable engine
            # rings (~80 GB/s each, serial per ring). The ACT-side stats
            # chunks (h >= 5) go first so the scalar engine's accumulate
            # passes start as early as possible; bn_stats chunks follow.
            engs = [nc.sync, nc.gpsimd, nc.scalar]
            order = [(h, ct) for h in (6, 7) for ct in range(CT)] + \
                    [(h, ct) for h in range(6) for ct in range(CT)]
            for i, (h, ct) in enumerate(order):
                cs = slice(128 * ct, 128 * (ct + 1))
                ds_ = slice(512 * h, 512 * (h + 1))
                engs[i % 3].dma_start(out=x_sb[ct][:, ds_], in_=xf[cs, ds_])
            for ct in range(CT):
                nc.scalar.dma_start(out=wT_sb[ct], in_=wT[128 * ct: 128 * (ct + 1), :])
                nc.sync.dma_start(out=pT_sb[ct], in_=pT[128 * ct: 128 * (ct + 1), :])
                nc.sync.dma_start(out=G_sb[ct], in_=Gm[ct])
                nc.sync.dma_start(out=P_sb[ct], in_=Pm[ct])
            nc.sync.dma_start(out=qb_sb, in_=qb)
            nc.sync.dma_start(out=id_sb, in_=idm)
            nc.sync.dma_start(out=pb_sb, in_=pb.rearrange("(t p) -> p t", p=128))
            nc.sync.dma_start(out=nw_sb, in_=nw.rearrange("(t p) -> p t", p=128))
            nc.sync.dma_start(out=nb_sb, in_=nb.rearrange("(t p) -> p t", p=128))

            # ones blocks of vT2: hh=0 -> d 64:128, hh=1 -> d 0:64
            for hh in range(2):
                nc.gpsimd.memset(
                    v2_view(64 * (1 + hh), [[512, MT], [256, 2], [1, 64]]),
                    1.0,
                )

            # ---------- x DMA chunks + GroupNorm statistics ----------
            with tc.tile_pool(name="warm", bufs=1, space="PSUM") as wmp, \
                 tc.tile_pool(name="gnps", bufs=1, space="PSUM") as gp, \
                 tc.tile_pool(name="stats", bufs=2) as stp:
                # PE warm-up on a dummy tile: runs during the x DMA so the
                # QKV matmuls start at 2.4 GHz instead of the throttled clock
                warm_ps = wmp.tile([128, 512], f32, name="warm_ps")
                for _ in range(48):
                    nc.tensor.matmul(
                        warm_ps, warm_sb[:, 0:128], warm_sb,
                        start=True, stop=True,
                    )
                # stats split: DVE bn_stats on cols 0:2560 of each
                # channel tile, ACT accumulate-sums on cols 2560:4096 (those
                # chunks land last; ACT is otherwise idle during the x DMA).
                # G carries 1/(32*N) so the group matmul sums raw totals.
                NSUB = 6
                stats = [
                    stp.tile([128, NSUB, 6], f32, tag=f"bnst{ct}", name=f"bnst{ct}")
                    for ct in range(CT)
                ]
                asum = singles.tile([128, CT, 1, 2], f32, tag="asum", name="asum")
                for ct in range(CT):
                    for i in range(NSUB):
                        ds_ = slice(512 * i, 512 * (i + 1))
                        nc.vector.bn_stats(
                            out=stats[ct][:, i, :], in_=x_sb[ct][:, ds_]
                        )
                # preload the Exp ACT table set while the x DMA streams
                nc.scalar.activation(
                    out=dm_sb, in_=eps_sb[0:GROUPS, :], func=AF.Exp,
                )
                ABLK = [(512 * NSUB, 1024)]
                for ct in range(CT):
                    for b, (off, w) in enumerate(ABLK):
                        ds_ = slice(off, off + w)
                        nc.scalar.activation(
                            out=scr_sb[:, 0:w], in_=x_sb[ct][:, ds_],
                            func=AF.Identity, accum_out=asum[:, ct, b, 0:1],
                        )
                        nc.scalar.activation(
                            out=scr_sb[:, 0:w], in_=x_sb[ct][:, ds_],
                            func=AF.Square, accum_out=asum[:, ct, b, 1:2],
                        )
                ND = 512.0 * NSUB  # samples covered by the bn_stats side
                gs_ps = gp.tile([GROUPS, 2], f32, tag="gs", name="gs")
                for ct in range(CT):
                    mv = stp.tile([128, 4], f32, tag="mv", name="mv")
                    nc.vector.bn_aggr(out=mv[:, 0:2], in_=stats[ct])
                    nc.vector.tensor_mul(
                        out=mv[:, 2:3], in0=mv[:, 0:1], in1=mv[:, 0:1]
                    )
                    nc.vector.tensor_add(
                        out=mv[:, 3:4], in0=mv[:, 1:2], in1=mv[:, 2:3]
                    )
                    # cstat = per-channel raw totals [sum(x), sum(x^2)]
                    cstat = stp.tile([128, 2], f32, tag="cstat", name="cstat")
                    ts_ = asum[:, ct, 0, :]
                    nc.vector.scalar_tensor_tensor(
                        out=cstat[:, 0:1], in0=mv[:, 0:1], scalar=ND,
                        in1=ts_[:, 0:1], op0=Alu.mult, op1=Alu.add,
                    )
                    nc.vector.scalar_tensor_tensor(
                        out=cstat[:, 1:2], in0=mv[:, 3:4], scalar=ND,
                        in1=ts_[:, 1:2], op0=Alu.mult, op1=Alu.add,
                    )
                    # group sums: gs[g, :] = sum_c G[c, g] * cstat[c, :]
                    nc.tensor.matmul(
                        gs_ps, G_sb[ct], cstat, start=(ct == 0), stop=(ct == CT - 1)
                    )
                # mr = [mean_g, rstd_g]
                nc.vector.tensor_copy(out=mr_sb[:, 0:1], in_=gs_ps[:, 0:1])
                gm2 = stp.tile([GROUPS, 1], f32, tag="gm2", name="gm2")
                nc.vector.tensor_mul(out=gm2, in0=mr_sb[:, 0:1], in1=mr_sb[:, 0:1])
                var_g = stp.tile([GROUPS, 1], f32, tag="varg", name="varg")
                nc.vector.scalar_tensor_tensor(
                    out=var_g, in0=gs_ps[:, 1:2], scalar=float(EPS),
                    in1=gm2, op0=Alu.add, op1=Alu.subtract,
                )
                # rstd = 1/sqrt(var) on DVE only: sqrt bit-trick seed,
                # fast reciprocal, then two Newton rsqrt refinements
                # (no ACT table-set switches)
                sqh = stp.tile([GROUPS, 1], f32, tag="sqh", name="sqh")
                sq0 = stp.tile([GROUPS, 1], f32, tag="sq0", name="sq0")
                nc.vector.tensor_scalar(
                    out=sqh.bitcast(mybir.dt.int32),
                    in0=var_g.bitcast(mybir.dt.int32),
                    scalar1=1, scalar2=None,
                    op0=Alu.logical_shift_right,
                )
                nc.vector.tensor_scalar(
                    out=sq0.bitcast(mybir.dt.int32),
                    in0=sqh.bitcast(mybir.dt.int32),
                    scalar1=0x1FBD1DF5, scalar2=None,
                    op0=Alu.add,
                )
                yv = rs_sb
                nc.vector.reciprocal_approx_fast(out=yv, in_=sq0)
                for it in range(2):
                    nt = stp.tile([GROUPS, 1], f32, tag=f"nt{it}", name="nt")
                    y2 = stp.tile([GROUPS, 1], f32, tag=f"y2{it}", name="y2")
                    nc.vector.tensor_mul(out=nt, in0=var_g, in1=yv)
                    nc.vector.tensor_mul(out=nt, in0=nt, in1=yv)
                    # nt = 1.5 - 0.5*var*y^2
                    nc.vector.tensor_scalar(
                        out=nt, in0=nt, scalar1=-0.5, scalar2=1.5,
                        op0=Alu.mult, op1=Alu.add,
                    )
                    dst = mr_sb[:, 1:2] if it == 1 else y2
                    nc.vector.tensor_mul(out=dst, in0=yv, in1=nt)
                    yv = y2
                # broadcast to channels; emit s and the W'-scale first so the
                # QKV matmuls unblock as early as possible, then t for biases
                pcs = []
                for ct in range(CT):
                    pc_ps = gp.tile([128, 2], f32, tag=f"pc{ct}", name="pc")
                    nc.tensor.matmul(pc_ps, P_sb[ct], mr_sb, start=True, stop=True)
                    nc.vector.tensor_mul(
                        out=s_sb[ct], in0=pc_ps[:, 1:2], in1=nw_sb[:, ct: ct + 1]
                    )
                    nc.vector.tensor_scalar(
                        out=wTs_sb[ct], in0=wT_sb[ct], scalar1=s_sb[ct],
                        scalar2=None, op0=Alu.mult,
                    )
                    pcs.append(pc_ps)
                for ct in range(CT):
                    tt = stp.tile([128, 1], f32, tag="tt", name="tt")
                    nc.vector.tensor_mul(out=tt, in0=pcs[ct][:, 0:1], in1=s_sb[ct])
                    nc.vector.tensor_sub(
                        out=t16_sb[:, ct: ct + 1], in0=nb_sb[:, ct: ct + 1], in1=tt
                    )
            # ---------- QKV projections ----------
            with tc.tile_pool(name="kqpsum", bufs=3, space="PSUM") as qp, \
                 tc.tile_pool(name="vtpsum", bufs=2, space="PSUM") as vp:
                # q first: unblocks the first attention iteration earliest
                for hp in range(2):
                    qps = qp.tile([128, NQ], f32, tag="kq", name="kq")
                    for chk in range(NQ // 512):
                        ns = slice(512 * chk, 512 * (chk + 1))
                        for ct in range(CT):
                            nc.tensor.matmul(
                                qps[:, ns],
                                wTs_sb[ct][:, 128 * hp: 128 * (hp + 1)],
                                x_sb[ct][:, ns],
                                start=(ct == 0), stop=(ct == CT - 1),
                            )
                    nc.scalar.activation(
                        out=q_sb[hp], in_=qps, func=AF.Identity,
                        bias=qb2_sb[:, hp: hp + 1],
                    )

                def emit_k(hp):
                    for chk in range(N // 1024):
                        ks_ = slice(1024 * chk, 1024 * (chk + 1))
                        kp = qp.tile([128, 1024], f32, tag="kq", name="kq")
                        for sub in range(2):
                            ns = slice(1024 * chk + 512 * sub,
                                       1024 * chk + 512 * (sub + 1))
                            for ct in range(CT):
                                nc.tensor.matmul(
                                    kp[:, 512 * sub: 512 * (sub + 1)],
                                    wTs_sb[ct][:, C + 128 * hp: C + 128 * (hp + 1)],
                                    x_sb[ct][:, ns],
                                    start=(ct == 0), stop=(ct == CT - 1),
                                )
                        if chk % 2 == 0:
                            nc.scalar.activation(
                                out=k_sb[hp][:, ks_], in_=kp, func=AF.Identity,
                                bias=qb2_sb[:, 2 + hp: 3 + hp],
                            )
                        else:
                            nc.vector.tensor_scalar(
                                out=k_sb[hp][:, ks_], in0=kp,
                                scalar1=qb2_sb[:, 2 + hp: 3 + hp], scalar2=None,
                                op0=Alu.add,
                            )

                emit_k(0)
                # bias fixups: qk: qb2 = qb + W_qk @ t; v (through proj,
                # valid because attention weights sum to 1):
                # pb2 = pb + P @ (W_v @ t)
                fixt = qp.tile([128, 1024], f32, tag="kq", name="fixt")
                fix_ps = fixt[:, 0:4]
                for j in range(4):
                    for ct in range(CT):
                        nc.tensor.matmul(
                            fix_ps[:, j: j + 1],
                            wT_sb[ct][:, 128 * j: 128 * (j + 1)],
                            t16_sb[:, ct: ct + 1],
                            start=(ct == 0), stop=(ct == CT - 1),
                        )
                u_ps = fixt[:, 4:6]
                for j in range(2):
                    for ct in range(CT):
                        nc.tensor.matmul(
                            u_ps[:, j: j + 1],
                            wT_sb[ct][:, 2 * C + 128 * j: 2 * C + 128 * (j + 1)],
                            t16_sb[:, ct: ct + 1],
                            start=(ct == 0), stop=(ct == CT - 1),
                        )
                nc.vector.tensor_add(out=qb2_sb, in0=qb_sb, in1=fix_ps)
                nc.vector.tensor_copy(out=u16_sb, in_=u_ps)
                pbf_ps = fixt[:, 6:8]
                for ot in range(2):
                    for ct in range(CT):
                        nc.tensor.matmul(
                            pbf_ps[:, ot: ot + 1],
                            pT_sb[ct][:, 128 * ot: 128 * (ot + 1)],
                            u16_sb[:, ct: ct + 1],
                            start=(ct == 0), stop=(ct == CT - 1),
                        )
                nc.vector.tensor_add(out=pb2_sb, in0=pb_sb, in1=pbf_ps)

                for mtp in range(MT // 2):
                    vps = vp.tile([128, 2 * C], f32, tag="vt", name="vt")
                    for j in range(2):
                        ms = slice(128 * (2 * mtp + j), 128 * (2 * mtp + j + 1))
                        for ct in range(CT):
                            nc.tensor.matmul(
                                vps[:, C * j: C * (j + 1)],
                                x_sb[ct][:, ms],
                                wTs_sb[ct][:, 2 * C: 3 * C],
                                start=(ct == 0), stop=(ct == CT - 1),
                            )
                    # strided f32->bf16 converts scattering the v channels
                    # into the vT2 layout (one per head pair: ISA caps APs at
                    # 3 free dims); v channel c = (hp, hh, dv), d-off 64*hh.
                    # (v bias is folded into the proj bias; valid because the
                    # attention weights sum to exactly 1.)
                    src = vps.rearrange(
                        "p (j hp hh dv) -> p hp j hh dv", j=2, hp=2, hh=2
                    )
                    for hp_ in range(2):
                        dst = v2_view(
                            1024 * mtp + 256 * hp_,
                            [[512, 2], [192, 2], [1, 64]],
                        )
                        if (2 * mtp + hp_) % 4 == 3:
                            nc.scalar.copy(out=dst, in_=src[:, hp_])
                        else:
                            nc.vector.tensor_copy(out=dst, in_=src[:, hp_])
                emit_k(1)

            # ---------- attention + proj ----------
            # PSUM budget (8 banks): scores pool [128,1024]x3 = 6 banks,
            # "acc" tag pool [128,512]x2 = 2 banks. Each accumulator bank
            # receives one combined AV+sigma DoubleRow-fp8 matmul stream
            # (M=128: 64 v columns + 64 ones columns; two key tiles per
            # instruction), so o and its softmax denominator land in
            # complementary partition halves of the same bank.
            #
            # Software pipeline: scores(gi) is emitted on the PE BEFORE
            # AV(gi-1), so the PE streams matmuls continuously while the
            # exp of gi-1 finishes on ACT/DVE. The proj of each query
            # chunk is deferred into the next unit's matmul stream
            # (borrowing a scores PSUM slot) so the PE never waits on the
            # normalization chain at unit boundaries.
            with tc.tile_pool(name="scps", bufs=3, space="PSUM") as scp, \
                 tc.tile_pool(name="accps", bufs=2, space="PSUM") as accp, \
                 tc.tile_pool(name="esba", bufs=4) as esa, \
                 tc.tile_pool(name="esbd", bufs=4) as esd, \
                 tc.tile_pool(name="osb", bufs=3) as osb, \
                 tc.tile_pool(name="outsb", bufs=2) as outsb:
                NG = MT // 2  # 16 score/AV groups of 2 key tiles

                def emit_proj(cn, on_pair):
                    ns = slice(512 * cn, 512 * (cn + 1))
                    prt = scp.tile([128, 1024], f32, tag="sc", name="pr")
                    for ot in range(CT):
                        pr = prt[:, 512 * ot: 512 * (ot + 1)]
                        for hp in range(2):
                            nc.tensor.matmul(
                                pr,
                                pT_sb[hp][:, 128 * ot: 128 * (ot + 1)],
                                on_pair[hp],
                                start=(hp == 0), stop=(hp == 1),
                            )
                    for ot in range(CT):
                        out_t = outsb.tile([128, 512], f32, tag="out", name="out")
                        nc.vector.scalar_tensor_tensor(
                            out=out_t, in0=prt[:, 512 * ot: 512 * (ot + 1)],
                            scalar=pb2_sb[:, ot: ot + 1],
                            in1=x_sb[ot][:, ns], op0=Alu.add, op1=Alu.add,
                        )
                        nc.sync.dma_start(
                            out=y[128 * ot: 128 * ot + 64, ns], in_=out_t[0:64, :]
                        )
                        nc.gpsimd.dma_start(
                            out=y[128 * ot + 64: 128 * (ot + 1), ns],
                            in_=out_t[64:128, :],
                        )

                pending = None
                on_store = [None, None]
                for cn in range(NQ // 512):
                    ns = slice(512 * cn, 512 * (cn + 1))
                    for hp in range(2):
                        # acc[hh]: even head: [o(0:64); sigma(64:128)]
                        #          odd head:  [sigma(0:64); o(64:128)]
                        acc = [accp.tile([128, 512], f32, tag="acc", name=f"acc{h}")
                               for h in range(2)]

                        def emit_av(gi, e_pair, hh_outer=False):
                            order = ([(j, hh) for hh in range(2) for j in range(2)]
                                     if hh_outer else
                                     [(j, hh) for j in range(2) for hh in range(2)])
                            for j, hh in order:
                                mt = 2 * gi + j
                                first = (gi == 0 and j == 0)
                                last = (gi == NG - 1 and j == 1)
                                nc.tensor.matmul(
                                    acc[hh],
                                    vT2_sb[:, 512 * mt + 256 * hp + 128 * hh:
                                           512 * mt + 256 * hp + 128 * (hh + 1)],
                                    e_pair[j][:, 512 * hh: 512 * (hh + 1)],
                                    start=first, stop=last,
                                )

                        prev_e = None
                        for gi in range(NG):
                            e_pair = []  # e_pair[j] = [e_h0_j | e_h1_j]
                            for j in range(2):
                                mt = 2 * gi + j
                                sc = scp.tile([128, 1024], f32, tag="sc", name="sc")
                                # one wave of four concurrent 64x64-tile
                                # matmuls (hh rows x key-half cols)
                                for kc in range(2):
                                    for hh in range(2):
                                        nc.tensor.matmul(
                                            sc[64 * kc: 64 * (kc + 1),
                                               512 * hh: 512 * (hh + 1)],
                                            k_sb[hp][64 * hh: 64 * (hh + 1),
                                                     128 * mt + 64 * kc:
                                                     128 * mt + 64 * (kc + 1)],
                                            q_sb[hp][64 * hh: 64 * (hh + 1), ns],
                                            start=True, stop=True,
                                            tile_position=(64 * hh, 64 * kc),
                                        )
                                # ACT (exact exp) takes j=0, DVE (Schraudolph
                                # bit-trick) takes j=1: each tile's exp starts
                                # right after its own wave, halving the PSUM
                                # slot-recycle latency that sets the cadence
                                last_unit = (cn == 1 and hp == 1)
                                on_act = (j == 0) or (last_unit and gi >= NG - 2)
                                if on_act:
                                    e = esa.tile([128, 1024], bf16, tag="e", name="ea")
                                    nc.scalar.activation(
                                        out=e, in_=sc, func=AF.Exp, scale=SCALE,
                                    )
                                else:
                                    e = esd.tile([128, 1024], bf16, tag="e", name="ed")
                                    nc.vector.tensor_scalar(
                                        out=e.bitcast(mybir.dt.int16),
                                        in0=sc,
                                        scalar1=EXP_A, scalar2=EXP_B,
                                        op0=Alu.mult, op1=Alu.add,
                                    )
                                e_pair.append(e)
                            if prev_e is not None:
                                emit_av(gi - 1, prev_e)
                            if gi == 6 and pending is not None:
                                emit_proj(*pending)
                                pending = None
                            prev_e = e_pair
                        emit_av(NG - 1, prev_e, hh_outer=True)
                        # normalize: sigma sits in the complementary partition
                        # half; approx-reciprocal the full tile (base_partition
                        # must be 0), DMA the sigma half across, multiply.
                        rec = osb.tile([128, 512], f32, tag="rec", name="rec")
                        recb = osb.tile([128, 512], f32, tag="recb", name="recb")
                        rec2 = osb.tile([128, 512], f32, tag="rec2", name="rec2")
                        on = osb.tile([128, 512], bf16, tag="on", name="on")
                        nc.vector.reciprocal_approx_fast(out=rec, in_=acc[0])
                        nc.sync.dma_start(out=rec2[0:32, :], in_=rec[64:96, :])
                        nc.gpsimd.dma_start(out=rec2[32:64, :], in_=rec[96:128, :])
                        nc.vector.tensor_mul(
                            out=on[0:64, :], in0=acc[0][0:64, :], in1=rec2[0:64, :]
                        )
                        nc.vector.reciprocal_approx_fast(out=recb, in_=acc[1])
                        nc.sync.dma_start(out=rec2[64:96, :], in_=recb[0:32, :])
                        nc.gpsimd.dma_start(out=rec2[96:128, :], in_=recb[32:64, :])
                        nc.vector.tensor_mul(
                            out=on[64:128, :], in0=acc[1][64:128, :],
                            in1=rec2[64:128, :],
                        )
                        on_store[hp] = on
                    pending = (cn, [on_store[0], on_store[1]])
                emit_proj(*pending)

    if finalize:
        nc.finalize()
    else:
        nc.compile()
    return nc


_NC_CACHE = None


def _get_nc():
    global _NC_CACHE
    if _NC_CACHE is None:
        _NC_CACHE = _build_nc()
    return _NC_CACHE


def _make_in_maps(x, norm_w, norm_b, qkv_w, qkv_b, proj_w, proj_b):
    import ml_dtypes

    bf16 = ml_dtypes.bfloat16
    xr = np.ascontiguousarray(x.reshape(B, C, N), dtype=np.float32)
    wTm = np.ascontiguousarray(qkv_w.astype(np.float32).T).astype(bf16)
    pTm = np.ascontiguousarray(proj_w.astype(np.float32).T).astype(bf16)
    # v bias is not applied on device (attention weights sum to 1, so its
    # contribution to the output is proj_w @ v_bias, folded in here)
    pbf = (proj_b.astype(np.float32)
           + proj_w.astype(np.float32) @ qkv_b[2 * C: 3 * C].astype(np.float32))
    qbm = np.stack(
        [qkv_b[0:128], qkv_b[128:256],
         qkv_b[C: C + 128], qkv_b[C + 128: 2 * C]], axis=1
    ).astype(np.float32)
    G = np.zeros((CT, 128, GROUPS), np.float32)
    P = np.zeros((CT, GROUPS, 128), np.float32)
    for ct in range(CT):
        for c in range(128):
            g = (128 * ct + c) // (C // GROUPS)
            G[ct, c, g] = 1.0 / ((C // GROUPS) * N)
            P[ct, g, c] = 1.0
    shared = {
        "idm": np.eye(128, dtype=np.float32).astype(bf16),
        "wT": wTm, "pT": pTm,
        "qb": np.ascontiguousarray(qbm), "pb": pbf,
        "nw": norm_w.astype(np.float32), "nb": norm_b.astype(np.float32),
        "Gm": G, "Pm": P,
    }
    in_maps = []
    for core in range(NCORES):
        b = core // 4
        qs = (core % 4) * NQ
        m = dict(shared)
        # rotate so this core's query columns come first; GroupNorm stats
        # and the attention key-reduction are column-order invariant
        m["xf"] = np.ascontiguousarray(
            np.roll(xr[b], -qs, axis=1)
        ).astype(bf16)
        in_maps.append(m)
    return in_maps


def kernel(x, norm_w, norm_b, qkv_w, qkv_b, proj_w, proj_b, _trace=False):
    from concourse import bass_utils

    nc = _get_nc()
    in_maps = _make_in_maps(x, norm_w, norm_b, qkv_w, qkv_b, proj_w, proj_b)
    res = bass_utils.run_bass_kernel_spmd(
        nc, in_maps, core_ids=list(range(NCORES)), trace=_trace
    )
    out = np.empty((B, C, N), np.float32)
    for core in range(NCORES):
        b = core // 4
        qs = (core % 4) * NQ
        out[b][:, qs: qs + NQ] = res.results[core]["y"]
    out = out.reshape(B, C, 16, 16, 16)
    if _trace:
        return out, res
    return out


# revision 26
# speedup vs baseline: 1.0092x; 1.0092x over previous
"""AttentionBlock3D Trainium2 kernel.

Module: GroupNorm(8 groups) -> 1x1x1 conv QKV -> 4-head attention over
N=4096 spatial positions (head_dim 64) -> 1x1x1 conv proj -> residual.
Shapes: x [2, 256, 16, 16, 16] f32.

Sharding (8 cores): batch (2) x query-range (4 chunks of 1024 rows).
Host rotates each core's x columns so its 1024 query columns come first
(GroupNorm stats and the attention key-sum are order-invariant), and
ships x as bf16 -- so there is no separate query slice DMA and no
on-device normalize pass.

Per core:
  - GroupNorm stats overlap the x DMA (x lands as 16 parallel-queue
    chunks over the three DMA-capable engine rings): bn_stats on DVE for
    the early chunks, Identity/Square accumulate passes on ACT for the
    late ones; rstd = 1/sqrt(var) entirely on DVE (sqrt bit-trick seed +
    fast reciprocal + 2 Newton steps) so the Exp ACT table set loads
    once and never switches.
  - The GN affine (s_c, t_c) is folded into the QKV weights on device:
    W' = W * s (per input channel), bias' = b + W @ t (tiny matmuls,
    deferred behind the k matmuls), so QKV reads raw bf16 x directly.
  - k, v for ALL 4096 keys (all heads), q only for its 1024 rows.
  - Attention is software-pipelined: per group gi (2 key tiles) the PE
    runs two 4-concurrent 64x64-tile score waves, one [128,1024] score
    tile per (gi, j) holding both half-heads side by side; ACT (exact
    exp) takes j=0 and DVE (Schraudolph bf16 bit-trick) takes j=1, each
    starting right after its own wave, which halves the PSUM
    slot-recycle latency. AV(gi-1) is emitted after scores(gi) so the
    PE streams continuously and HAM stays at full clock.
  - Softmax is unnormalized; the denominator comes from ones-columns in
    the AV matmul (free: matmul cost is N-cycles regardless of M) and is
    divided out after attention*V (partition-shift via split DMAs).
  - proj + bias + residual per query chunk, deferred into the next
    unit's matmul stream (borrowing a scores PSUM slot) so the PE never
    waits on the normalization chain at unit boundaries.
Gather on host is pure concatenation.

Layouts on device (per core):
  x  [C=256, N] bf16 -> 2 channel-tiles of [128, N]
  k_sb[pair]      [128, 4096] bf16: partitions = [head 2p (64); 2p+1]
  q_sb[pair]      [128, 1024] bf16: same head-pair packing
  vT2_sb          [128, 32*512] bf16: partitions = key rows m; per key
                  tile mt four 128-col blocks (hp, hh), hh=0 [v|ones],
                  hh=1 [ones|v]
  scores^T        PSUM [m 128, n 512] via row-tiled (K=64) matmul pairs
  attention out   acc[hh] [128, 512]: o and its softmax denominator land
                  in complementary partition halves of the same bank
"""

import math
import numpy as np

B = 2
C = 256
NH = 4
GROUPS = 8
EPS = 1e-5
N = 16 * 16 * 16  # 4096
HD = C // NH      # 64
NQ = N // 4       # 1024 query rows per core
NCORES = 8
CT = 2            # channel tiles of 128
MT = N // 128     # 32 key tiles
SCALE = HD ** -0.5
# Schraudolph bf16 exp: bits_i16(round(A*x + B)) viewed as bf16 ~= exp(x)
EXP_A = SCALE * 128.0 / math.log(2.0)
EXP_B = 127.0 * 128.0 - 0.0430 * 128.0


def _build_nc(finalize=True):
    import concourse.bacc as bacc
    import concourse.bass as bass
    import concourse.mybir as mybir
    from concourse.tile import TileContext

    f32 = mybir.dt.float32
    bf16 = mybir.dt.bfloat16
    fp8 = mybir.dt.float8e4
    DR = mybir.MatmulPerfMode.DoubleRow
    Alu = mybir.AluOpType
    AF = mybir.ActivationFunctionType

    nc = bacc.Bacc("TRN2", debug=False)

    xf = nc.dram_tensor("xf", [C, N], bf16, kind="ExternalInput").ap()
    wT = nc.dram_tensor("wT", [C, 3 * C], bf16, kind="ExternalInput").ap()
    pT = nc.dram_tensor("pT", [C, C], bf16, kind="ExternalInput").ap()
    qb = nc.dram_tensor("qb", [128, 4], f32, kind="ExternalInput").ap()
    pb = nc.dram_tensor("pb", [C], f32, kind="ExternalInput").ap()
    nw = nc.dram_tensor("nw", [C], f32, kind="ExternalInput").ap()
    nb = nc.dram_tensor("nb", [C], f32, kind="ExternalInput").ap()
    Gm = nc.dram_tensor("Gm", [CT, 128, GROUPS], f32, kind="ExternalInput").ap()
    Pm = nc.dram_tensor("Pm", [CT, GROUPS, 128], f32, kind="ExternalInput").ap()
    y = nc.dram_tensor("y", [C, NQ], f32, kind="ExternalOutput").ap()

    with TileContext(nc) as tc:
        import contextlib

        est = contextlib.ExitStack()
        with est:
            singles = est.enter_context(tc.tile_pool(name="singles", bufs=1))

            # ---------- persistent SBUF tiles ----------
            x_sb = [singles.tile([128, N], bf16, tag=f"x{ct}", name=f"x{ct}") for ct in range(CT)]
            wT_sb = [singles.tile([128, 3 * C], bf16, tag=f"wT{ct}", name=f"wT{ct}") for ct in range(CT)]
            wTs_sb = [singles.tile([128, 3 * C], bf16, tag=f"wTs{ct}", name=f"wTs{ct}") for ct in range(CT)]
            pT_sb = [singles.tile([128, C], bf16, tag=f"pT{ct}", name=f"pT{ct}") for ct in range(CT)]
            k_sb = [singles.tile([128, N], bf16, tag=f"k{p}", name=f"k{p}") for p in range(2)]
            q_sb = [singles.tile([128, NQ], bf16, tag=f"q{p}", name=f"q{p}") for p in range(2)]
            # vT2: [p, mt(32):512, hp(2):256, hh(2):128, d(128):1]
            # d = [v(64)|ones(64)] for hh=0, [ones(64)|v(64)] for hh=1
            vT2_sb = singles.tile([128, MT * 512], bf16, tag="vT2", name="vT2")
            qb_sb = singles.tile([128, 4], f32, tag="qb", name="qb")
            qb2_sb = singles.tile([128, 4], f32, tag="qb2", name="qb2")
            pb_sb = singles.tile([128, 2], f32, tag="pb", name="pb")
            pb2_sb = singles.tile([128, 2], f32, tag="pb2", name="pb2")
            nw_sb = singles.tile([128, CT], f32, tag="nw", name="nw")
            nb_sb = singles.tile([128, CT], f32, tag="nb", name="nb")
            G_sb = [singles.tile([128, GROUPS], f32, tag=f"G{ct}", name=f"G{ct}") for ct in range(CT)]
            P_sb = [singles.tile([GROUPS, 128], f32, tag=f"P{ct}", name=f"P{ct}") for ct in range(CT)]
            eps_sb = singles.tile([128, 1], f32, tag="eps", name="eps")
            s_sb = [singles.tile([128, 1], f32, tag=f"s{ct}", name=f"s{ct}") for ct in range(CT)]
            t16_sb = singles.tile([128, CT], bf16, tag="t16", name="t16")
            u16_sb = singles.tile([128, 2], bf16, tag="u16", name="u16")
            mr_sb = singles.tile([GROUPS, 2], f32, tag="mr", name="mr")
            rs_sb = singles.tile([GROUPS, 1], f32, tag="rs", name="rs")
            dm_sb = singles.tile([GROUPS, 1], f32, tag="dm", name="dm")
            warm_sb = singles.tile([128, 512], bf16, tag="warm", name="warm")
            scr_sb = singles.tile([128, 1024], f32, tag="scr", name="scr")

            def v2_view(off, dims):
                return bass.AP(
                    tensor=vT2_sb.tensor,
                    offset=vT2_sb.offset + off,
                    ap=[list(vT2_sb.ap[0])] + [list(d) for d in dims],
                )

            # ---------- t=0: x DMA first (8 parallel queues, issued from
            # two engines so descriptor writes don't serialize), then weights
            nc.vector.memset(warm_sb, 0.0)
            nc.vector.memset(eps_sb, EPS)
            # x in 16 [128, 512] chunks over the three DMA-capable engine
            # rings (~80 GB/s each, serial per ring). The ACT-side stats
            # chunks (h >= 5) go first so the scalar engine's accumulate
            # passes start as early as possible; bn_stats chunks follow.
            engs = [nc.sync, nc.gpsimd, nc.scalar]
            order = [(h, ct) for h in (6, 7) for ct in range(CT)] + \
                    [(h, ct) for h in range(6) for ct in range(CT)]
            for i, (h, ct) in enumerate(order):
                cs = slice(128 * ct, 128 * (ct + 1))
                ds_ = slice(512 * h, 512 * (h + 1))
                engs[i % 3].dma_start(out=x_sb[ct][:, ds_], in_=xf[cs, ds_])
            for ct in range(CT):
                nc.scalar.dma_start(out=wT_sb[ct], in_=wT[128 * ct: 128 * (ct + 1), :])
                nc.sync.dma_start(out=pT_sb[ct], in_=pT[128 * ct: 128 * (ct + 1), :])
                nc.sync.dma_start(out=G_sb[ct], in_=Gm[ct])
                nc.sync.dma_start(out=P_sb[ct], in_=Pm[ct])
            nc.sync.dma_start(out=qb_sb, in_=qb)
            nc.sync.dma_start(out=pb_sb, in_=pb.rearrange("(t p) -> p t", p=128))
            nc.sync.dma_start(out=nw_sb, in_=nw.rearrange("(t p) -> p t", p=128))
            nc.sync.dma_start(out=nb_sb, in_=nb.rearrange("(t p) -> p t", p=128))

            # ones blocks of vT2: hh=0 -> d 64:128, hh=1 -> d 0:64
            for hh in range(2):
                nc.gpsimd.memset(
                    v2_view(64 * (1 + hh), [[512, MT], [256, 2], [1, 64]]),
                    1.0,
                )

            # ---------- x DMA chunks + GroupNorm statistics ----------
            with tc.tile_pool(name="warm", bufs=1, space="PSUM") as wmp, \
                 tc.tile_pool(name="gnps", bufs=1, space="PSUM") as gp, \
                 tc.tile_pool(name="stats", bufs=2) as stp:
                # PE warm-up on a dummy tile: runs during the x DMA so the
                # QKV matmuls start at 2.4 GHz instead of the throttled clock
                warm_ps = wmp.tile([128, 512], f32, name="warm_ps")
                for _ in range(48):
                    nc.tensor.matmul(
                        warm_ps, warm_sb[:, 0:128], warm_sb,
                        start=True, stop=True,
                    )
                # stats split: DVE bn_stats on cols 0:2560 of each
                # channel tile, ACT accumulate-sums on cols 2560:4096 (those
                # chunks land last; ACT is otherwise idle during the x DMA).
                # G carries 1/(32*N) so the group matmul sums raw totals.
                NSUB = 6
                stats = [
                    stp.tile([128, NSUB, 6], f32, tag=f"bnst{ct}", name=f"bnst{ct}")
                    for ct in range(CT)
                ]
                asum = singles.tile([128, CT, 1, 2], f32, tag="asum", name="asum")
                for ct in range(CT):
                    for i in range(NSUB):
                        ds_ = slice(512 * i, 512 * (i + 1))
                        nc.vector.bn_stats(
                            out=stats[ct][:, i, :], in_=x_sb[ct][:, ds_]
                        )
                # preload the Exp ACT table set while the x DMA streams
                nc.scalar.activation(
                    out=dm_sb, in_=eps_sb[0:GROUPS, :], func=AF.Exp,
                )
                ABLK = [(512 * NSUB, 1024)]
                for ct in range(CT):
                    for b, (off, w) in enumerate(ABLK):
                        ds_ = slice(off, off + w)
                        nc.scalar.activation(
                            out=scr_sb[:, 0:w], in_=x_sb[ct][:, ds_],
                            func=AF.Identity, accum_out=asum[:, ct, b, 0:1],
                        )
                        nc.scalar.activation(
                            out=scr_sb[:, 0:w], in_=x_sb[ct][:, ds_],
                            func=AF.Square, accum_out=asum[:, ct, b, 1:2],
                        )
                ND = 512.0 * NSUB  # samples covered by the bn_stats side
                gs_ps = gp.tile([GROUPS, 2], f32, tag="gs", name="gs")
                for ct in range(CT):
                    mv = stp.tile([128, 4], f32, tag="mv", name="mv")
                    nc.vector.bn_aggr(out=mv[:, 0:2], in_=stats[ct])
                    nc.vector.tensor_mul(
                        out=mv[:, 2:3], in0=mv[:, 0:1], in1=mv[:, 0:1]
                    )
                    nc.vector.tensor_add(
                        out=mv[:, 3:4], in0=mv[:, 1:2], in1=mv[:, 2:3]
                    )
                    # cstat = per-channel raw totals [sum(x), sum(x^2)]
                    cstat = stp.tile([128, 2], f32, tag="cstat", name="cstat")
                    ts_ = asum[:, ct, 0, :]
                    nc.vector.scalar_tensor_tensor(
                        out=cstat[:, 0:1], in0=mv[:, 0:1], scalar=ND,
                        in1=ts_[:, 0:1], op0=Alu.mult, op1=Alu.add,
                    )
                    nc.vector.scalar_tensor_tensor(
                        out=cstat[:, 1:2], in0=mv[:, 3:4], scalar=ND,
                        in1=ts_[:, 1:2], op0=Alu.mult, op1=Alu.add,
                    )
                    # group sums: gs[g, :] = sum_c G[c, g] * cstat[c, :]
                    nc.tensor.matmul(
                        gs_ps, G_sb[ct], cstat, start=(ct == 0), stop=(ct == CT - 1)
                    )
                # mr = [mean_g, rstd_g]
                nc.vector.tensor_copy(out=mr_sb[:, 0:1], in_=gs_ps[:, 0:1])
                gm2 = stp.tile([GROUPS, 1], f32, tag="gm2", name="gm2")
                nc.vector.tensor_mul(out=gm2, in0=mr_sb[:, 0:1], in1=mr_sb[:, 0:1])
                var_g = stp.tile([GROUPS, 1], f32, tag="varg", name="varg")
                nc.vector.scalar_tensor_tensor(
                    out=var_g, in0=gs_ps[:, 1:2], scalar=float(EPS),
                    in1=gm2, op0=Alu.add, op1=Alu.subtract,
                )
                # rstd = 1/sqrt(var) on DVE only: sqrt bit-trick seed,
                # fast reciprocal, then two Newton rsqrt refinements
                # (no ACT table-set switches)
                sqh = stp.tile([GROUPS, 1], f32, tag="sqh", name="sqh")
                sq0 = stp.tile([GROUPS, 1], f32, tag="sq0", name="sq0")
                nc.vector.tensor_scalar(
                    out=sqh.bitcast(mybir.dt.int32),
                    in0=var_g.bitcast(mybir.dt.int32),
                    scalar1=1, scalar2=None,
                    op0=Alu.logical_shift_right,
                )
                nc.vector.tensor_scalar(
                    out=sq0.bitcast(mybir.dt.int32),
                    in0=sqh.bitcast(mybir.dt.int32),
                    scalar1=0x1FBD1DF5, scalar2=None,
                    op0=Alu.add,
                )
                yv = rs_sb
                nc.vector.reciprocal_approx_fast(out=yv, in_=sq0)
                for it in range(2):
                    nt = stp.tile([GROUPS, 1], f32, tag=f"nt{it}", name="nt")
                    y2 = stp.tile([GROUPS, 1], f32, tag=f"y2{it}", name="y2")
                    nc.vector.tensor_mul(out=nt, in0=var_g, in1=yv)
                    nc.vector.tensor_mul(out=nt, in0=nt, in1=yv)
                    # nt = 1.5 - 0.5*var*y^2
                    nc.vector.tensor_scalar(
                        out=nt, in0=nt, scalar1=-0.5, scalar2=1.5,
                        op0=Alu.mult, op1=Alu.add,
                    )
                    dst = mr_sb[:, 1:2] if it == 1 else y2
                    nc.vector.tensor_mul(out=dst, in0=yv, in1=nt)
                    yv = y2
                # broadcast to channels; emit s and the W'-scale first so the
                # QKV matmuls unblock as early as possible, then t for biases
                pcs = []
                for ct in range(CT):
                    pc_ps = gp.tile([128, 2], f32, tag=f"pc{ct}", name="pc")
                    nc.tensor.matmul(pc_ps, P_sb[ct], mr_sb, start=True, stop=True)
                    nc.vector.tensor_mul(
                        out=s_sb[ct], in0=pc_ps[:, 1:2], in1=nw_sb[:, ct: ct + 1]
                    )
                    nc.vector.tensor_scalar(
                        out=wTs_sb[ct], in0=wT_sb[ct], scalar1=s_sb[ct],
                        scalar2=None, op0=Alu.mult,
                    )
                    pcs.append(pc_ps)
                for ct in range(CT):
                    tt = stp.tile([128, 1], f32, tag="tt", name="tt")
                    nc.vector.tensor_mul(out=tt, in0=pcs[ct][:, 0:1], in1=s_sb[ct])
                    nc.vector.tensor_sub(
                        out=t16_sb[:, ct: ct + 1], in0=nb_sb[:, ct: ct + 1], in1=tt
                    )
            # ---------- QKV projections ----------
            with tc.tile_pool(name="kqpsum", bufs=3, space="PSUM") as qp, \
                 tc.tile_pool(name="vtpsum", bufs=2, space="PSUM") as vp:
                # q first: unblocks the first attention iteration earliest
                for hp in range(2):
                    qps = qp.tile([128, NQ], f32, tag="kq", name="kq")
                    for chk in range(NQ // 512):
                        ns = slice(512 * chk, 512 * (chk + 1))
                        for ct in range(CT):
                            nc.tensor.matmul(
                                qps[:, ns],
                                wTs_sb[ct][:, 128 * hp: 128 * (hp + 1)],
                                x_sb[ct][:, ns],
                                start=(ct == 0), stop=(ct == CT - 1),
                            )
                    nc.scalar.activation(
                        out=q_sb[hp], in_=qps, func=AF.Identity,
                        bias=qb2_sb[:, hp: hp + 1],
                    )

                def emit_k(hp):
                    for chk in range(N // 1024):
                        ks_ = slice(1024 * chk, 1024 * (chk + 1))
                        kp = qp.tile([128, 1024], f32, tag="kq", name="kq")
                        for sub in range(2):
                            ns = slice(1024 * chk + 512 * sub,
                                       1024 * chk + 512 * (sub + 1))
                            for ct in range(CT):
                                nc.tensor.matmul(
                                    kp[:, 512 * sub: 512 * (sub + 1)],
                                    wTs_sb[ct][:, C + 128 * hp: C + 128 * (hp + 1)],
                                    x_sb[ct][:, ns],
                                    start=(ct == 0), stop=(ct == CT - 1),
                                )
                        if chk % 2 == 0:
                            nc.scalar.activation(
                                out=k_sb[hp][:, ks_], in_=kp, func=AF.Identity,
                                bias=qb2_sb[:, 2 + hp: 3 + hp],
                            )
                        else:
                            nc.vector.tensor_scalar(
                                out=k_sb[hp][:, ks_], in0=kp,
                                scalar1=qb2_sb[:, 2 + hp: 3 + hp], scalar2=None,
                                op0=Alu.add,
                            )

                emit_k(0)
                # bias fixups: qk: qb2 = qb + W_qk @ t; v (through proj,
                # valid because attention weights sum to 1):
                # pb2 = pb + P @ (W_v @ t)
                fixt = qp.tile([128, 1024], f32, tag="kq", name="fixt")
                fix_ps = fixt[:, 0:4]
                for j in range(4):
                    for ct in range(CT):
                        nc.tensor.matmul(
                            fix_ps[:, j: j + 1],
                            wT_sb[ct][:, 128 * j: 128 * (j + 1)],
                            t16_sb[:, ct: ct + 1],
                            start=(ct == 0), stop=(ct == CT - 1),
                        )
                u_ps = fixt[:, 4:6]
                for j in range(2):
                    for ct in range(CT):
                        nc.tensor.matmul(
                            u_ps[:, j: j + 1],
                            wT_sb[ct][:, 2 * C + 128 * j: 2 * C + 128 * (j + 1)],
                            t16_sb[:, ct: ct + 1],
                            start=(ct == 0), stop=(ct == CT - 1),
                        )
                nc.vector.tensor_add(out=qb2_sb, in0=qb_sb, in1=fix_ps)
                nc.vector.tensor_copy(out=u16_sb, in_=u_ps)
                pbf_ps = fixt[:, 6:8]
                for ot in range(2):
                    for ct in range(CT):
                        nc.tensor.matmul(
                            pbf_ps[:, ot: ot + 1],
                            pT_sb[ct][:, 128 * ot: 128 * (ot + 1)],
                            u16_sb[:, ct: ct + 1],
                            start=(ct == 0), stop=(ct == CT - 1),
                        )
                nc.vector.tensor_add(out=pb2_sb, in0=pb_sb, in1=pbf_ps)

                for mtp in range(MT // 2):
                    vps = vp.tile([128, 2 * C], f32, tag="vt", name="vt")
                    for j in range(2):
                        ms = slice(128 * (2 * mtp + j), 128 * (2 * mtp + j + 1))
                        for ct in range(CT):
                            nc.tensor.matmul(
                                vps[:, C * j: C * (j + 1)],
                                x_sb[ct][:, ms],
                                wTs_sb[ct][:, 2 * C: 3 * C],
                                start=(ct == 0), stop=(ct == CT - 1),
                            )
                    # strided f32->bf16 converts scattering the v channels
                    # into the vT2 layout (one per head pair: ISA caps APs at
                    # 3 free dims); v channel c = (hp, hh, dv), d-off 64*hh.
                    # (v bias is folded into the proj bias; valid because the
                    # attention weights sum to exactly 1.)
                    src = vps.rearrange(
                        "p (j hp hh dv) -> p hp j hh dv", j=2, hp=2, hh=2
                    )
                    for hp_ in range(2):
                        dst = v2_view(
                            1024 * mtp + 256 * hp_,
                            [[512, 2], [192, 2], [1, 64]],
                        )
                        if (2 * mtp + hp_) % 4 == 3:
                            nc.scalar.copy(out=dst, in_=src[:, hp_])
                        else:
                            nc.vector.tensor_copy(out=dst, in_=src[:, hp_])
                emit_k(1)

            # ---------- attention + proj ----------
            # PSUM budget (8 banks): scores pool [128,1024]x3 = 6 banks,
            # "acc" tag pool [128,512]x2 = 2 banks. Each accumulator bank
            # receives one combined AV+sigma DoubleRow-fp8 matmul stream
            # (M=128: 64 v columns + 64 ones columns; two key tiles per
            # instruction), so o and its softmax denominator land in
            # complementary partition halves of the same bank.
            #
            # Software pipeline: scores(gi) is emitted on the PE BEFORE
            # AV(gi-1), so the PE streams matmuls continuously while the
            # exp of gi-1 finishes on ACT/DVE. The proj of each query
            # chunk is deferred into the next unit's matmul stream
            # (borrowing a scores PSUM slot) so the PE never waits on the
            # normalization chain at unit boundaries.
            with tc.tile_pool(name="scps", bufs=3, space="PSUM") as scp, \
                 tc.tile_pool(name="accps", bufs=2, space="PSUM") as accp, \
                 tc.tile_pool(name="esba", bufs=4) as esa, \
                 tc.tile_pool(name="esbd", bufs=4) as esd, \
                 tc.tile_pool(name="osb", bufs=3) as osb, \
                 tc.tile_pool(name="outsb", bufs=2) as outsb:
                NG = MT // 2  # 16 score/AV groups of 2 key tiles

                def emit_proj(cn, on_pair):
                    ns = slice(512 * cn, 512 * (cn + 1))
                    prt = scp.tile([128, 1024], f32, tag="sc", name="pr")
                    for ot in range(CT):
                        pr = prt[:, 512 * ot: 512 * (ot + 1)]
                        for hp in range(2):
                            nc.tensor.matmul(
                                pr,
                                pT_sb[hp][:, 128 * ot: 128 * (ot + 1)],
                                on_pair[hp],
                                start=(hp == 0), stop=(hp == 1),
                            )
                    for ot in range(CT):
                        out_t = outsb.tile([128, 512], f32, tag="out", name="out")
                        nc.vector.scalar_tensor_tensor(
                            out=out_t, in0=prt[:, 512 * ot: 512 * (ot + 1)],
                            scalar=pb2_sb[:, ot: ot + 1],
                            in1=x_sb[ot][:, ns], op0=Alu.add, op1=Alu.add,
                        )
                        nc.sync.dma_start(
                            out=y[128 * ot: 128 * ot + 64, ns], in_=out_t[0:64, :]
                        )
                        nc.gpsimd.dma_start(
                            out=y[128 * ot + 64: 128 * (ot + 1), ns],
                            in_=out_t[64:128, :],
                        )

                pending = None
                on_store = [None, None]
                for cn in range(NQ // 512):
                    ns = slice(512 * cn, 512 * (cn + 1))
                    for hp in range(2):
                        # acc[hh]: even head: [o(0:64); sigma(64:128)]
                        #          odd head:  [sigma(0:64); o(64:128)]
                        acc = [accp.tile([128, 512], f32, tag="acc", name=f"acc{h}")
                               for h in range(2)]

                        def emit_av(gi, e_pair, hh_outer=False):
                            order = ([(j, hh) for hh in range(2) for j in range(2)]
                                     if hh_outer else
                                     [(j, hh) for j in range(2) for hh in range(2)])
                            for j, hh in order:
                                mt = 2 * gi + j
                                first = (gi == 0 and j == 0)
                                last = (gi == NG - 1 and j == 1)
                                nc.tensor.matmul(
                                    acc[hh],
                                    vT2_sb[:, 512 * mt + 256 * hp + 128 * hh:
                                           512 * mt + 256 * hp + 128 * (hh + 1)],
                                    e_pair[j][:, 512 * hh: 512 * (hh + 1)],
                                    start=first, stop=last,
                                )

                        prev_e = None
                        for gi in range(NG):
                            e_pair = []  # e_pair[j] = [e_h0_j | e_h1_j]
                            for j in range(2):
                                mt = 2 * gi + j
                                sc = scp.tile([128, 1024], f32, tag="sc", name="sc")
                                # one wave of four concurrent 64x64-tile
                                # matmuls (hh rows x key-half cols)
                                for kc in range(2):
                                    for hh in range(2):
                                        nc.tensor.matmul(
                                            sc[64 * kc: 64 * (kc + 1),
                                               512 * hh: 512 * (hh + 1)],
                                            k_sb[hp][64 * hh: 64 * (hh + 1),
                                                     128 * mt + 64 * kc:
                                                     128 * mt + 64 * (kc + 1)],
                                            q_sb[hp][64 * hh: 64 * (hh + 1), ns],
                                            start=True, stop=True,
                                            tile_position=(64 * hh, 64 * kc),
                                        )
                                # ACT (exact exp) takes j=0, DVE (Schraudolph
                                # bit-trick) takes j=1: each tile's exp starts
                                # right after its own wave, halving the PSUM
                                # slot-recycle latency that sets the cadence
                                last_unit = (cn == 1 and hp == 1)
                                on_act = (j == 0) or (last_unit and gi >= NG - 2)
                                if on_act:
                                    e = esa.tile([128, 1024], bf16, tag="e", name="ea")
                                    nc.scalar.activation(
                                        out=e, in_=sc, func=AF.Exp, scale=SCALE,
                                    )
                                else:
                                    e = esd.tile([128, 1024], bf16, tag="e", name="ed")
                                    nc.vector.tensor_scalar(
                                        out=e.bitcast(mybir.dt.int16),
                                        in0=sc,
                                        scalar1=EXP_A, scalar2=EXP_B,
                                        op0=Alu.mult, op1=Alu.add,
                                    )
                                e_pair.append(e)
                            if prev_e is not None:
                                emit_av(gi - 1, prev_e)
                            if gi == 6 and pending is not None:
                                emit_proj(*pending)
                                pending = None
                            prev_e = e_pair
                        emit_av(NG - 1, prev_e, hh_outer=True)
                        # normalize: sigma sits in the complementary partition
                        # half; approx-reciprocal the full tile (base_partition
                        # must be 0), DMA the sigma half across, multiply.
                        rec = osb.tile([128, 512], f32, tag="rec", name="rec")
                        recb = osb.tile([128, 512], f32, tag="recb", name="recb")
                        rec2 = osb.tile([128, 512], f32, tag="rec2", name="rec2")
                        on = osb.tile([128, 512], bf16, tag="on", name="on")
                        nc.vector.reciprocal_approx_fast(out=rec, in_=acc[0])
                        nc.sync.dma_start(out=rec2[0:32, :], in_=rec[64:96, :])
                        nc.gpsimd.dma_start(out=rec2[32:64, :], in_=rec[96:128, :])
                        nc.vector.tensor_mul(
                            out=on[0:64, :], in0=acc[0][0:64, :], in1=rec2[0:64, :]
                        )
                        nc.vector.reciprocal_approx_fast(out=recb, in_=acc[1])
                        nc.sync.dma_start(out=rec2[64:96, :], in_=recb[0:32, :])
                        nc.gpsimd.dma_start(out=rec2[96:128, :], in_=recb[32:64, :])
                        nc.vector.tensor_mul(
                            out=on[64:128, :], in0=acc[1][64:128, :],
                            in1=rec2[64:128, :],
                        )
                        on_store[hp] = on
                    pending = (cn, [on_store[0], on_store[1]])
                emit_proj(*pending)

    if finalize:
        nc.finalize()
    else:
        nc.compile()
    return nc


_NC_CACHE = None


def _get_nc():
    global _NC_CACHE
    if _NC_CACHE is None:
        _NC_CACHE = _build_nc()
    return _NC_CACHE


def _make_in_maps(x, norm_w, norm_b, qkv_w, qkv_b, proj_w, proj_b):
    import ml_dtypes

    bf16 = ml_dtypes.bfloat16
    xr = np.ascontiguousarray(x.reshape(B, C, N), dtype=np.float32)
    wTm = np.ascontiguousarray(qkv_w.astype(np.float32).T).astype(bf16)
    pTm = np.ascontiguousarray(proj_w.astype(np.float32).T).astype(bf16)
    # v bias is not applied on device (attention weights sum to 1, so its
    # contribution to the output is proj_w @ v_bias, folded in here)
    pbf = (proj_b.astype(np.float32)
           + proj_w.astype(np.float32) @ qkv_b[2 * C: 3 * C].astype(np.float32))
    qbm = np.stack(
        [qkv_b[0:128], qkv_b[128:256],
         qkv_b[C: C + 128], qkv_b[C + 128: 2 * C]], axis=1
    ).astype(np.float32)
    G = np.zeros((CT, 128, GROUPS), np.float32)
    P = np.zeros((CT, GROUPS, 128), np.float32)
    for ct in range(CT):
        for c in range(128):
            g = (128 * ct + c) // (C // GROUPS)
            G[ct, c, g] = 1.0 / ((C // GROUPS) * N)
            P[ct, g, c] = 1.0
    shared = {
        "wT": wTm, "pT": pTm,
        "qb": np.ascontiguousarray(qbm), "pb": pbf,
        "nw": norm_w.astype(np.float32), "nb": norm_b.astype(np.float32),
        "Gm": G, "Pm": P,
    }
    in_maps = []
    for core in range(NCORES):
        b = core // 4
        qs = (core % 4) * NQ
        m = dict(shared)
        # rotate so this core's query columns come first; GroupNorm stats
        # and the attention key-reduction are column-order invariant
        m["xf"] = np.ascontiguousarray(
            np.roll(xr[b], -qs, axis=1)
        ).astype(bf16)
        in_maps.append(m)
    return in_maps


def kernel(x, norm_w, norm_b, qkv_w, qkv_b, proj_w, proj_b, _trace=False):
    from concourse import bass_utils

    nc = _get_nc()
    in_maps = _make_in_maps(x, norm_w, norm_b, qkv_w, qkv_b, proj_w, proj_b)
    res = bass_utils.run_bass_kernel_spmd(
        nc, in_maps, core_ids=list(range(NCORES)), trace=_trace
    )
    out = np.empty((B, C, N), np.float32)
    for core in range(NCORES):
        b = core // 4
        qs = (core % 4) * NQ
        out[b][:, qs: qs + NQ] = res.results[core]["y"]
    out = out.reshape(B, C, 16, 16, 16)
    if _trace:
        return out, res
    return out


# revision 27
# speedup vs baseline: 1.0146x; 1.0054x over previous
"""AttentionBlock3D Trainium2 kernel.

Module: GroupNorm(8 groups) -> 1x1x1 conv QKV -> 4-head attention over
N=4096 spatial positions (head_dim 64) -> 1x1x1 conv proj -> residual.
Shapes: x [2, 256, 16, 16, 16] f32.

Sharding (8 cores): batch (2) x query-range (4 chunks of 1024 rows).
Host rotates each core's x columns so its 1024 query columns come first
(GroupNorm stats and the attention key-sum are order-invariant), and
ships x as bf16 -- so there is no separate query slice DMA and no
on-device normalize pass.

Per core:
  - GroupNorm stats overlap the x DMA (x lands as 16 parallel-queue
    chunks over the three DMA-capable engine rings): bn_stats on DVE for
    the early chunks, Identity/Square accumulate passes on ACT for the
    late ones; rstd = 1/sqrt(var) entirely on DVE (sqrt bit-trick seed +
    fast reciprocal + 2 Newton steps) so the Exp ACT table set loads
    once and never switches.
  - The GN affine (s_c, t_c) is folded into the QKV weights on device:
    W' = W * s (per input channel), bias' = b + W @ t (tiny matmuls,
    deferred behind the k matmuls), so QKV reads raw bf16 x directly.
  - k, v for ALL 4096 keys (all heads), q only for its 1024 rows.
  - Attention is software-pipelined: per group gi (2 key tiles) the PE
    runs two 4-concurrent 64x64-tile score waves, one [128,1024] score
    tile per (gi, j) holding both half-heads side by side; ACT (exact
    exp) takes j=0 and DVE (Schraudolph bf16 bit-trick) takes j=1, each
    starting right after its own wave, which halves the PSUM
    slot-recycle latency. AV(gi-1) is emitted after scores(gi) so the
    PE streams continuously and HAM stays at full clock.
  - Softmax is unnormalized; the denominator comes from ones-columns in
    the AV matmul (free: matmul cost is N-cycles regardless of M) and is
    divided out after attention*V (partition-shift via split DMAs).
  - proj + bias + residual per query chunk, deferred into the next
    unit's matmul stream (borrowing a scores PSUM slot) so the PE never
    waits on the normalization chain at unit boundaries.
Gather on host is pure concatenation.

Layouts on device (per core):
  x  [C=256, N] bf16 -> 2 channel-tiles of [128, N]
  k_sb[pair]      [128, 4096] bf16: partitions = [head 2p (64); 2p+1]
  q_sb[pair]      [128, 1024] bf16: same head-pair packing
  vT2_sb          [128, 32*512] bf16: partitions = key rows m; per key
                  tile mt four 128-col blocks (hp, hh), hh=0 [v|ones],
                  hh=1 [ones|v]
  scores^T        PSUM [m 128, n 512] via row-tiled (K=64) matmul pairs
  attention out   acc[hh] [128, 512]: o and its softmax denominator land
                  in complementary partition halves of the same bank
"""

import math
import numpy as np

B = 2
C = 256
NH = 4
GROUPS = 8
EPS = 1e-5
N = 16 * 16 * 16  # 4096
HD = C // NH      # 64
NQ = N // 4       # 1024 query rows per core
NCORES = 8
CT = 2            # channel tiles of 128
MT = N // 128     # 32 key tiles
SCALE = HD ** -0.5
# Schraudolph bf16 exp: bits_i16(round(A*x + B)) viewed as bf16 ~= exp(x)
EXP_A = SCALE * 128.0 / math.log(2.0)
EXP_B = 127.0 * 128.0 - 0.0430 * 128.0


def _build_nc(finalize=True):
    import concourse.bacc as bacc
    import concourse.bass as bass
    import concourse.mybir as mybir
    from concourse.tile import TileContext

    f32 = mybir.dt.float32
    bf16 = mybir.dt.bfloat16
    fp8 = mybir.dt.float8e4
    DR = mybir.MatmulPerfMode.DoubleRow
    Alu = mybir.AluOpType
    AF = mybir.ActivationFunctionType

    nc = bacc.Bacc("TRN2", debug=False)

    xf = nc.dram_tensor("xf", [C, N], bf16, kind="ExternalInput").ap()
    wT = nc.dram_tensor("wT", [C, 3 * C], bf16, kind="ExternalInput").ap()
    pT = nc.dram_tensor("pT", [C, C], bf16, kind="ExternalInput").ap()
    qb = nc.dram_tensor("qb", [128, 4], f32, kind="ExternalInput").ap()
    pb = nc.dram_tensor("pb", [C], f32, kind="ExternalInput").ap()
    nw = nc.dram_tensor("nw", [C], f32, kind="ExternalInput").ap()
    nb = nc.dram_tensor("nb", [C], f32, kind="ExternalInput").ap()
    Gm = nc.dram_tensor("Gm", [CT, 128, GROUPS], f32, kind="ExternalInput").ap()
    Pm = nc.dram_tensor("Pm", [CT, GROUPS, 128], f32, kind="ExternalInput").ap()
    y = nc.dram_tensor("y", [C, NQ], f32, kind="ExternalOutput").ap()

    with TileContext(nc) as tc:
        import contextlib

        est = contextlib.ExitStack()
        with est:
            singles = est.enter_context(tc.tile_pool(name="singles", bufs=1))

            # ---------- persistent SBUF tiles ----------
            x_sb = [singles.tile([128, N], bf16, tag=f"x{ct}", name=f"x{ct}") for ct in range(CT)]
            wT_sb = [singles.tile([128, 3 * C], bf16, tag=f"wT{ct}", name=f"wT{ct}") for ct in range(CT)]
            wTs_sb = [singles.tile([128, 3 * C], bf16, tag=f"wTs{ct}", name=f"wTs{ct}") for ct in range(CT)]
            pT_sb = [singles.tile([128, C], bf16, tag=f"pT{ct}", name=f"pT{ct}") for ct in range(CT)]
            k_sb = [singles.tile([128, N], bf16, tag=f"k{p}", name=f"k{p}") for p in range(2)]
            q_sb = [singles.tile([128, NQ], bf16, tag=f"q{p}", name=f"q{p}") for p in range(2)]
            # vT2: [p, mt(32):512, hp(2):256, hh(2):128, d(128):1]
            # d = [v(64)|ones(64)] for hh=0, [ones(64)|v(64)] for hh=1
            vT2_sb = singles.tile([128, MT * 512], bf16, tag="vT2", name="vT2")
            qb_sb = singles.tile([128, 4], f32, tag="qb", name="qb")
            qb2_sb = singles.tile([128, 4], f32, tag="qb2", name="qb2")
            pb_sb = singles.tile([128, 2], f32, tag="pb", name="pb")
            pb2_sb = singles.tile([128, 2], f32, tag="pb2", name="pb2")
            nw_sb = singles.tile([128, CT], f32, tag="nw", name="nw")
            nb_sb = singles.tile([128, CT], f32, tag="nb", name="nb")
            G_sb = [singles.tile([128, GROUPS], f32, tag=f"G{ct}", name=f"G{ct}") for ct in range(CT)]
            P_sb = [singles.tile([GROUPS, 128], f32, tag=f"P{ct}", name=f"P{ct}") for ct in range(CT)]
            eps_sb = singles.tile([128, 1], f32, tag="eps", name="eps")
            s_sb = [singles.tile([128, 1], f32, tag=f"s{ct}", name=f"s{ct}") for ct in range(CT)]
            t16_sb = singles.tile([128, CT], bf16, tag="t16", name="t16")
            u16_sb = singles.tile([128, 2], bf16, tag="u16", name="u16")
            mr_sb = singles.tile([GROUPS, 2], f32, tag="mr", name="mr")
            rs_sb = singles.tile([GROUPS, 1], f32, tag="rs", name="rs")
            dm_sb = singles.tile([GROUPS, 1], f32, tag="dm", name="dm")
            warm_sb = singles.tile([128, 512], bf16, tag="warm", name="warm")
            scr_sb = singles.tile([128, 1024], f32, tag="scr", name="scr")

            def v2_view(off, dims):
                return bass.AP(
                    tensor=vT2_sb.tensor,
                    offset=vT2_sb.offset + off,
                    ap=[list(vT2_sb.ap[0])] + [list(d) for d in dims],
                )

            # ---------- t=0: x DMA first (8 parallel queues, issued from
            # two engines so descriptor writes don't serialize), then weights
            nc.vector.memset(warm_sb, 0.0)
            nc.vector.memset(eps_sb, EPS)
            # x in 16 [128, 512] chunks over the three DMA-capable engine
            # rings (~80 GB/s each, serial per ring). The ACT-side stats
            # chunks (h >= 5) go first so the scalar engine's accumulate
            # passes start as early as possible; bn_stats chunks follow.
            engs = [nc.sync, nc.gpsimd, nc.scalar]
            order = [(h, ct) for h in (6, 7) for ct in range(CT)] + \
                    [(h, ct) for h in range(6) for ct in range(CT)]
            for i, (h, ct) in enumerate(order):
                cs = slice(128 * ct, 128 * (ct + 1))
                ds_ = slice(512 * h, 512 * (h + 1))
                engs[i % 3].dma_start(out=x_sb[ct][:, ds_], in_=xf[cs, ds_])
            for ct in range(CT):
                nc.scalar.dma_start(out=wT_sb[ct], in_=wT[128 * ct: 128 * (ct + 1), :])
                nc.sync.dma_start(out=pT_sb[ct], in_=pT[128 * ct: 128 * (ct + 1), :])
                nc.sync.dma_start(out=G_sb[ct], in_=Gm[ct])
                nc.sync.dma_start(out=P_sb[ct], in_=Pm[ct])
            nc.sync.dma_start(out=qb_sb, in_=qb)
            nc.sync.dma_start(out=pb_sb, in_=pb.rearrange("(t p) -> p t", p=128))
            nc.sync.dma_start(out=nw_sb, in_=nw.rearrange("(t p) -> p t", p=128))
            nc.sync.dma_start(out=nb_sb, in_=nb.rearrange("(t p) -> p t", p=128))

            # ones blocks of vT2: hh=0 -> d 64:128, hh=1 -> d 0:64
            for hh in range(2):
                nc.gpsimd.memset(
                    v2_view(64 * (1 + hh), [[512, MT], [256, 2], [1, 64]]),
                    1.0,
                )

            # ---------- x DMA chunks + GroupNorm statistics ----------
            with tc.tile_pool(name="warm", bufs=1, space="PSUM") as wmp, \
                 tc.tile_pool(name="gnps", bufs=1, space="PSUM") as gp, \
                 tc.tile_pool(name="stats", bufs=2) as stp:
                # PE warm-up on a dummy tile: runs during the x DMA so the
                # QKV matmuls start at 2.4 GHz instead of the throttled clock
                warm_ps = wmp.tile([128, 512], f32, name="warm_ps")
                for _ in range(58):
                    nc.tensor.matmul(
                        warm_ps, warm_sb[:, 0:128], warm_sb,
                        start=True, stop=True,
                    )
                # stats split: DVE bn_stats on cols 0:2560 of each
                # channel tile, ACT accumulate-sums on cols 2560:4096 (those
                # chunks land last; ACT is otherwise idle during the x DMA).
                # G carries 1/(32*N) so the group matmul sums raw totals.
                NSUB = 6
                stats = [
                    stp.tile([128, NSUB, 6], f32, tag=f"bnst{ct}", name=f"bnst{ct}")
                    for ct in range(CT)
                ]
                asum = singles.tile([128, CT, 1, 2], f32, tag="asum", name="asum")
                for ct in range(CT):
                    for i in range(NSUB):
                        ds_ = slice(512 * i, 512 * (i + 1))
                        nc.vector.bn_stats(
                            out=stats[ct][:, i, :], in_=x_sb[ct][:, ds_]
                        )
                # preload the Exp ACT table set while the x DMA streams
                nc.scalar.activation(
                    out=dm_sb, in_=eps_sb[0:GROUPS, :], func=AF.Exp,
                )
                ABLK = [(512 * NSUB, 1024)]
                for ct in range(CT):
                    for b, (off, w) in enumerate(ABLK):
                        ds_ = slice(off, off + w)
                        nc.scalar.activation(
                            out=scr_sb[:, 0:w], in_=x_sb[ct][:, ds_],
                            func=AF.Identity, accum_out=asum[:, ct, b, 0:1],
                        )
                        nc.scalar.activation(
                            out=scr_sb[:, 0:w], in_=x_sb[ct][:, ds_],
                            func=AF.Square, accum_out=asum[:, ct, b, 1:2],
                        )
                ND = 512.0 * NSUB  # samples covered by the bn_stats side
                gs_ps = gp.tile([GROUPS, 2], f32, tag="gs", name="gs")
                for ct in range(CT):
                    mv = stp.tile([128, 4], f32, tag="mv", name="mv")
                    nc.vector.bn_aggr(out=mv[:, 0:2], in_=stats[ct])
                    nc.vector.tensor_mul(
                        out=mv[:, 2:3], in0=mv[:, 0:1], in1=mv[:, 0:1]
                    )
                    nc.vector.tensor_add(
                        out=mv[:, 3:4], in0=mv[:, 1:2], in1=mv[:, 2:3]
                    )
                    # cstat = per-channel raw totals [sum(x), sum(x^2)]
                    cstat = stp.tile([128, 2], f32, tag="cstat", name="cstat")
                    ts_ = asum[:, ct, 0, :]
                    nc.vector.scalar_tensor_tensor(
                        out=cstat[:, 0:1], in0=mv[:, 0:1], scalar=ND,
                        in1=ts_[:, 0:1], op0=Alu.mult, op1=Alu.add,
                    )
                    nc.vector.scalar_tensor_tensor(
                        out=cstat[:, 1:2], in0=mv[:, 3:4], scalar=ND,
                        in1=ts_[:, 1:2], op0=Alu.mult, op1=Alu.add,
                    )
                    # group sums: gs[g, :] = sum_c G[c, g] * cstat[c, :]
                    nc.tensor.matmul(
                        gs_ps, G_sb[ct], cstat, start=(ct == 0), stop=(ct == CT - 1)
                    )
                # mr = [mean_g, rstd_g]
                nc.vector.tensor_copy(out=mr_sb[:, 0:1], in_=gs_ps[:, 0:1])
                gm2 = stp.tile([GROUPS, 1], f32, tag="gm2", name="gm2")
                nc.vector.tensor_mul(out=gm2, in0=mr_sb[:, 0:1], in1=mr_sb[:, 0:1])
                var_g = stp.tile([GROUPS, 1], f32, tag="varg", name="varg")
                nc.vector.scalar_tensor_tensor(
                    out=var_g, in0=gs_ps[:, 1:2], scalar=float(EPS),
                    in1=gm2, op0=Alu.add, op1=Alu.subtract,
                )
                # rstd = 1/sqrt(var) on DVE only: sqrt bit-trick seed,
                # fast reciprocal, then two Newton rsqrt refinements
                # (no ACT table-set switches)
                sqh = stp.tile([GROUPS, 1], f32, tag="sqh", name="sqh")
                sq0 = stp.tile([GROUPS, 1], f32, tag="sq0", name="sq0")
                nc.vector.tensor_scalar(
                    out=sqh.bitcast(mybir.dt.int32),
                    in0=var_g.bitcast(mybir.dt.int32),
                    scalar1=1, scalar2=None,
                    op0=Alu.logical_shift_right,
                )
                nc.vector.tensor_scalar(
                    out=sq0.bitcast(mybir.dt.int32),
                    in0=sqh.bitcast(mybir.dt.int32),
                    scalar1=0x1FBD1DF5, scalar2=None,
                    op0=Alu.add,
                )
                yv = rs_sb
                nc.vector.reciprocal_approx_fast(out=yv, in_=sq0)
                for it in range(2):
                    nt = stp.tile([GROUPS, 1], f32, tag=f"nt{it}", name="nt")
                    y2 = stp.tile([GROUPS, 1], f32, tag=f"y2{it}", name="y2")
                    nc.vector.tensor_mul(out=nt, in0=var_g, in1=yv)
                    nc.vector.tensor_mul(out=nt, in0=nt, in1=yv)
                    # nt = 1.5 - 0.5*var*y^2
                    nc.vector.tensor_scalar(
                        out=nt, in0=nt, scalar1=-0.5, scalar2=1.5,
                        op0=Alu.mult, op1=Alu.add,
                    )
                    dst = mr_sb[:, 1:2] if it == 1 else y2
                    nc.vector.tensor_mul(out=dst, in0=yv, in1=nt)
                    yv = y2
                # broadcast to channels; emit s and the W'-scale first so the
                # QKV matmuls unblock as early as possible, then t for biases
                pcs = []
                for ct in range(CT):
                    pc_ps = gp.tile([128, 2], f32, tag=f"pc{ct}", name="pc")
                    nc.tensor.matmul(pc_ps, P_sb[ct], mr_sb, start=True, stop=True)
                    nc.vector.tensor_mul(
                        out=s_sb[ct], in0=pc_ps[:, 1:2], in1=nw_sb[:, ct: ct + 1]
                    )
                    nc.vector.tensor_scalar(
                        out=wTs_sb[ct], in0=wT_sb[ct], scalar1=s_sb[ct],
                        scalar2=None, op0=Alu.mult,
                    )
                    pcs.append(pc_ps)
                for ct in range(CT):
                    tt = stp.tile([128, 1], f32, tag="tt", name="tt")
                    nc.vector.tensor_mul(out=tt, in0=pcs[ct][:, 0:1], in1=s_sb[ct])
                    nc.vector.tensor_sub(
                        out=t16_sb[:, ct: ct + 1], in0=nb_sb[:, ct: ct + 1], in1=tt
                    )
            # ---------- QKV projections ----------
            with tc.tile_pool(name="kqpsum", bufs=3, space="PSUM") as qp, \
                 tc.tile_pool(name="vtpsum", bufs=2, space="PSUM") as vp:
                # q first: unblocks the first attention iteration earliest
                for hp in range(2):
                    qps = qp.tile([128, NQ], f32, tag="kq", name="kq")
                    for chk in range(NQ // 512):
                        ns = slice(512 * chk, 512 * (chk + 1))
                        for ct in range(CT):
                            nc.tensor.matmul(
                                qps[:, ns],
                                wTs_sb[ct][:, 128 * hp: 128 * (hp + 1)],
                                x_sb[ct][:, ns],
                                start=(ct == 0), stop=(ct == CT - 1),
                            )
                    nc.scalar.activation(
                        out=q_sb[hp], in_=qps, func=AF.Identity,
                        bias=qb2_sb[:, hp: hp + 1],
                    )

                def emit_k(hp):
                    for chk in range(N // 1024):
                        ks_ = slice(1024 * chk, 1024 * (chk + 1))
                        kp = qp.tile([128, 1024], f32, tag="kq", name="kq")
                        for sub in range(2):
                            ns = slice(1024 * chk + 512 * sub,
                                       1024 * chk + 512 * (sub + 1))
                            for ct in range(CT):
                                nc.tensor.matmul(
                                    kp[:, 512 * sub: 512 * (sub + 1)],
                                    wTs_sb[ct][:, C + 128 * hp: C + 128 * (hp + 1)],
                                    x_sb[ct][:, ns],
                                    start=(ct == 0), stop=(ct == CT - 1),
                                )
                        if chk % 2 == 0:
                            nc.scalar.activation(
                                out=k_sb[hp][:, ks_], in_=kp, func=AF.Identity,
                                bias=qb2_sb[:, 2 + hp: 3 + hp],
                            )
                        else:
                            nc.vector.tensor_scalar(
                                out=k_sb[hp][:, ks_], in0=kp,
                                scalar1=qb2_sb[:, 2 + hp: 3 + hp], scalar2=None,
                                op0=Alu.add,
                            )

                emit_k(0)
                # bias fixups: qk: qb2 = qb + W_qk @ t; v (through proj,
                # valid because attention weights sum to 1):
                # pb2 = pb + P @ (W_v @ t)
                fixt = qp.tile([128, 1024], f32, tag="kq", name="fixt")
                fix_ps = fixt[:, 0:4]
                for j in range(4):
                    for ct in range(CT):
                        nc.tensor.matmul(
                            fix_ps[:, j: j + 1],
                            wT_sb[ct][:, 128 * j: 128 * (j + 1)],
                            t16_sb[:, ct: ct + 1],
                            start=(ct == 0), stop=(ct == CT - 1),
                        )
                u_ps = fixt[:, 4:6]
                for j in range(2):
                    for ct in range(CT):
                        nc.tensor.matmul(
                            u_ps[:, j: j + 1],
                            wT_sb[ct][:, 2 * C + 128 * j: 2 * C + 128 * (j + 1)],
                            t16_sb[:, ct: ct + 1],
                            start=(ct == 0), stop=(ct == CT - 1),
                        )
                nc.vector.tensor_add(out=qb2_sb, in0=qb_sb, in1=fix_ps)
                nc.vector.tensor_copy(out=u16_sb, in_=u_ps)
                pbf_ps = fixt[:, 6:8]
                for ot in range(2):
                    for ct in range(CT):
                        nc.tensor.matmul(
                            pbf_ps[:, ot: ot + 1],
                            pT_sb[ct][:, 128 * ot: 128 * (ot + 1)],
                            u16_sb[:, ct: ct + 1],
                            start=(ct == 0), stop=(ct == CT - 1),
                        )
                nc.vector.tensor_add(out=pb2_sb, in0=pb_sb, in1=pbf_ps)

                for mtp in range(MT // 2):
                    vps = vp.tile([128, 2 * C], f32, tag="vt", name="vt")
                    for j in range(2):
                        ms = slice(128 * (2 * mtp + j), 128 * (2 * mtp + j + 1))
                        for ct in range(CT):
                            nc.tensor.matmul(
                                vps[:, C * j: C * (j + 1)],
                                x_sb[ct][:, ms],
                                wTs_sb[ct][:, 2 * C: 3 * C],
                                start=(ct == 0), stop=(ct == CT - 1),
                            )
                    # strided f32->bf16 converts scattering the v channels
                    # into the vT2 layout (one per head pair: ISA caps APs at
                    # 3 free dims); v channel c = (hp, hh, dv), d-off 64*hh.
                    # (v bias is folded into the proj bias; valid because the
                    # attention weights sum to exactly 1.)
                    src = vps.rearrange(
                        "p (j hp hh dv) -> p hp j hh dv", j=2, hp=2, hh=2
                    )
                    for hp_ in range(2):
                        dst = v2_view(
                            1024 * mtp + 256 * hp_,
                            [[512, 2], [192, 2], [1, 64]],
                        )
                        if (2 * mtp + hp_) % 4 == 3:
                            nc.scalar.copy(out=dst, in_=src[:, hp_])
                        else:
                            nc.vector.tensor_copy(out=dst, in_=src[:, hp_])
                emit_k(1)

            # ---------- attention + proj ----------
            # PSUM budget (8 banks): scores pool [128,1024]x3 = 6 banks,
            # "acc" tag pool [128,512]x2 = 2 banks. Each accumulator bank
            # receives one combined AV+sigma DoubleRow-fp8 matmul stream
            # (M=128: 64 v columns + 64 ones columns; two key tiles per
            # instruction), so o and its softmax denominator land in
            # complementary partition halves of the same bank.
            #
            # Software pipeline: scores(gi) is emitted on the PE BEFORE
            # AV(gi-1), so the PE streams matmuls continuously while the
            # exp of gi-1 finishes on ACT/DVE. The proj of each query
            # chunk is deferred into the next unit's matmul stream
            # (borrowing a scores PSUM slot) so the PE never waits on the
            # normalization chain at unit boundaries.
            with tc.tile_pool(name="scps", bufs=3, space="PSUM") as scp, \
                 tc.tile_pool(name="accps", bufs=2, space="PSUM") as accp, \
                 tc.tile_pool(name="esba", bufs=4) as esa, \
                 tc.tile_pool(name="esbd", bufs=4) as esd, \
                 tc.tile_pool(name="osb", bufs=3) as osb, \
                 tc.tile_pool(name="outsb", bufs=2) as outsb:
                NG = MT // 2  # 16 score/AV groups of 2 key tiles

                def emit_proj(cn, on_pair):
                    ns = slice(512 * cn, 512 * (cn + 1))
                    prt = scp.tile([128, 1024], f32, tag="sc", name="pr")
                    for ot in range(CT):
                        pr = prt[:, 512 * ot: 512 * (ot + 1)]
                        for hp in range(2):
                            nc.tensor.matmul(
                                pr,
                                pT_sb[hp][:, 128 * ot: 128 * (ot + 1)],
                                on_pair[hp],
                                start=(hp == 0), stop=(hp == 1),
                            )
                    for ot in range(CT):
                        out_t = outsb.tile([128, 512], f32, tag="out", name="out")
                        nc.vector.scalar_tensor_tensor(
                            out=out_t, in0=prt[:, 512 * ot: 512 * (ot + 1)],
                            scalar=pb2_sb[:, ot: ot + 1],
                            in1=x_sb[ot][:, ns], op0=Alu.add, op1=Alu.add,
                        )
                        nc.sync.dma_start(
                            out=y[128 * ot: 128 * ot + 64, ns], in_=out_t[0:64, :]
                        )
                        nc.gpsimd.dma_start(
                            out=y[128 * ot + 64: 128 * (ot + 1), ns],
                            in_=out_t[64:128, :],
                        )

                pending = None
                on_store = [None, None]
                for cn in range(NQ // 512):
                    ns = slice(512 * cn, 512 * (cn + 1))
                    for hp in range(2):
                        # acc[hh]: even head: [o(0:64); sigma(64:128)]
                        #          odd head:  [sigma(0:64); o(64:128)]
                        acc = [accp.tile([128, 512], f32, tag="acc", name=f"acc{h}")
                               for h in range(2)]

                        def emit_av(gi, e_pair, hh_outer=False):
                            order = ([(j, hh) for hh in range(2) for j in range(2)]
                                     if hh_outer else
                                     [(j, hh) for j in range(2) for hh in range(2)])
                            for j, hh in order:
                                mt = 2 * gi + j
                                first = (gi == 0 and j == 0)
                                last = (gi == NG - 1 and j == 1)
                                nc.tensor.matmul(
                                    acc[hh],
                                    vT2_sb[:, 512 * mt + 256 * hp + 128 * hh:
                                           512 * mt + 256 * hp + 128 * (hh + 1)],
                                    e_pair[j][:, 512 * hh: 512 * (hh + 1)],
                                    start=first, stop=last,
                                )

                        prev_e = None
                        for gi in range(NG):
                            e_pair = []  # e_pair[j] = [e_h0_j | e_h1_j]
                            for j in range(2):
                                mt = 2 * gi + j
                                sc = scp.tile([128, 1024], f32, tag="sc", name="sc")
                                # one wave of four concurrent 64x64-tile
                                # matmuls (hh rows x key-half cols)
                                for kc in range(2):
                                    for hh in range(2):
                                        nc.tensor.matmul(
                                            sc[64 * kc: 64 * (kc + 1),
                                               512 * hh: 512 * (hh + 1)],
                                            k_sb[hp][64 * hh: 64 * (hh + 1),
                                                     128 * mt + 64 * kc:
                                                     128 * mt + 64 * (kc + 1)],
                                            q_sb[hp][64 * hh: 64 * (hh + 1), ns],
                                            start=True, stop=True,
                                            tile_position=(64 * hh, 64 * kc),
                                        )
                                # ACT (exact exp) takes j=0, DVE (Schraudolph
                                # bit-trick) takes j=1: each tile's exp starts
                                # right after its own wave, halving the PSUM
                                # slot-recycle latency that sets the cadence
                                # j=0 -> ACT; two j=1 tiles per unit also
                                # go to ACT so DVE keeps slack for the norm +
                                # residual work in its queue
                                on_act = (j == 0) or gi in (7, NG - 1)
                                if on_act:
                                    e = esa.tile([128, 1024], bf16, tag="e", name="ea")
                                    nc.scalar.activation(
                                        out=e, in_=sc, func=AF.Exp, scale=SCALE,
                                    )
                                else:
                                    e = esd.tile([128, 1024], bf16, tag="e", name="ed")
                                    nc.vector.tensor_scalar(
                                        out=e.bitcast(mybir.dt.int16),
                                        in0=sc,
                                        scalar1=EXP_A, scalar2=EXP_B,
                                        op0=Alu.mult, op1=Alu.add,
                                    )
                                e_pair.append(e)
                            if prev_e is not None:
                                emit_av(gi - 1, prev_e)
                            if gi == 6 and pending is not None:
                                emit_proj(*pending)
                                pending = None
                            prev_e = e_pair
                        emit_av(NG - 1, prev_e, hh_outer=True)
                        # normalize: sigma sits in the complementary partition
                        # half; approx-reciprocal the full tile (base_partition
                        # must be 0), DMA the sigma half across, multiply.
                        rec = osb.tile([128, 512], f32, tag="rec", name="rec")
                        recb = osb.tile([128, 512], f32, tag="recb", name="recb")
                        rec2 = osb.tile([128, 512], f32, tag="rec2", name="rec2")
                        on = osb.tile([128, 512], bf16, tag="on", name="on")
                        nc.vector.reciprocal_approx_fast(out=rec, in_=acc[0])
                        nc.sync.dma_start(out=rec2[0:32, :], in_=rec[64:96, :])
                        nc.gpsimd.dma_start(out=rec2[32:64, :], in_=rec[96:128, :])
                        nc.vector.tensor_mul(
                            out=on[0:64, :], in0=acc[0][0:64, :], in1=rec2[0:64, :]
                        )
                        nc.vector.reciprocal_approx_fast(out=recb, in_=acc[1])
                        nc.sync.dma_start(out=rec2[64:96, :], in_=recb[0:32, :])
                        nc.gpsimd.dma_start(out=rec2[96:128, :], in_=recb[32:64, :])
                        nc.vector.tensor_mul(
                            out=on[64:128, :], in0=acc[1][64:128, :],
                            in1=rec2[64:128, :],
                        )
                        on_store[hp] = on
                    pending = (cn, [on_store[0], on_store[1]])
                emit_proj(*pending)

    if finalize:
        nc.finalize()
    else:
        nc.compile()
    return nc


_NC_CACHE = None


def _get_nc():
    global _NC_CACHE
    if _NC_CACHE is None:
        _NC_CACHE = _build_nc()
    return _NC_CACHE


def _make_in_maps(x, norm_w, norm_b, qkv_w, qkv_b, proj_w, proj_b):
    import ml_dtypes

    bf16 = ml_dtypes.bfloat16
    xr = np.ascontiguousarray(x.reshape(B, C, N), dtype=np.float32)
    wTm = np.ascontiguousarray(qkv_w.astype(np.float32).T).astype(bf16)
    pTm = np.ascontiguousarray(proj_w.astype(np.float32).T).astype(bf16)
    # v bias is not applied on device (attention weights sum to 1, so its
    # contribution to the output is proj_w @ v_bias, folded in here)
    pbf = (proj_b.astype(np.float32)
           + proj_w.astype(np.float32) @ qkv_b[2 * C: 3 * C].astype(np.float32))
    qbm = np.stack(
        [qkv_b[0:128], qkv_b[128:256],
         qkv_b[C: C + 128], qkv_b[C + 128: 2 * C]], axis=1
    ).astype(np.float32)
    G = np.zeros((CT, 128, GROUPS), np.float32)
    P = np.zeros((CT, GROUPS, 128), np.float32)
    for ct in range(CT):
        for c in range(128):
            g = (128 * ct + c) // (C // GROUPS)
            G[ct, c, g] = 1.0 / ((C // GROUPS) * N)
            P[ct, g, c] = 1.0
    shared = {
        "wT": wTm, "pT": pTm,
        "qb": np.ascontiguousarray(qbm), "pb": pbf,
        "nw": norm_w.astype(np.float32), "nb": norm_b.astype(np.float32),
        "Gm": G, "Pm": P,
    }
    in_maps = []
    for core in range(NCORES):
        b = core // 4
        qs = (core % 4) * NQ
        m = dict(shared)
        # rotate so this core's query columns come first; GroupNorm stats
        # and the attention key-reduction are column-order invariant
        m["xf"] = np.ascontiguousarray(
            np.roll(xr[b], -qs, axis=1)
        ).astype(bf16)
        in_maps.append(m)
    return in_maps


def kernel(x, norm_w, norm_b, qkv_w, qkv_b, proj_w, proj_b, _trace=False):
    from concourse import bass_utils

    nc = _get_nc()
    in_maps = _make_in_maps(x, norm_w, norm_b, qkv_w, qkv_b, proj_w, proj_b)
    res = bass_utils.run_bass_kernel_spmd(
        nc, in_maps, core_ids=list(range(NCORES)), trace=_trace
    )
    out = np.empty((B, C, N), np.float32)
    for core in range(NCORES):
        b = core // 4
        qs = (core % 4) * NQ
        out[b][:, qs: qs + NQ] = res.results[core]["y"]
    out = out.reshape(B, C, 16, 16, 16)
    if _trace:
        return out, res
    return out
